# revision 12
# baseline (speedup 1.0000x reference)
# Trainium2 Bass kernel for the ContractiveREN forward pass.
#
# Math summary (matches the reference nn.Module):
#   derived params from X, Y (host, float64):
#     H = X^T X + eps I;  F=H31, B1=H32, Lam=diag(H22)/2,
#     D11=-tril(H22,-1), C1=-H21, E=(H11+a*H33+Y-Y^T)/2
#   per step t (device):
#     at = Lam^-1 (C1 x_t + D12 u_t)
#     w solves w = tanh(at + Dt w), Dt = Lam^-1 D11 (strictly lower)
#     x' = E^-1 (F x + B1 w + B2 u)          (folded: FE x + B1E w + B2E u)
#     y  = C2 x' + D21 w + D22 u             (folded: YX x + YW w + YU u)
#
# The strictly-lower-triangular tanh recurrence is solved with KFP dense
# fixed-point iterations w <- tanh(at + Dt w); convergence to below f32
# noise was verified empirically (k=16 -> rel err ~3e-7 end to end).
#
# To keep the serial dependency chain uniform (16 matmul->tanh hops per
# step and nothing else), at_{t+1} is computed directly from
# (x_t, w_t, u_t, u_{t+1}) via host-folded weights:
#   at_{t+1} = (C1t FE) x_t + (C1t B1E) w_t + (C1t B2E) u_t + D12t u_{t+1}
# so the x materialization (PSUM->SBUF copy) is off the critical path.
#
# All matmul operands are bitcast to float32r: fp32 matmuls lower to two
# PE passes (two LDWEIGHTS+MATMUL pairs) while float32r is single-pass,
# which halves the tensor-engine instruction stream.
#
# Sharding: data-parallel over batch, 8 cores x 32 batch elements. All
# device tensors keep batch in the free dimension (transposed layouts),
# parameters are replicated.

import numpy as np

import concourse.bacc as bacc
import concourse.mybir as mybir
import concourse.tile as tile
from concourse.bass_utils import run_bass_kernel_spmd

B, T = 256, 1024
IN_DIM, OUT_DIM = 32, 32
N_STATE, Q = 128, 128
EPS = 1e-3
ALPHA = 1.0
NCORES = 8
BL = B // NCORES          # local batch per core (free dim)
NSTEP = T - 1             # last scan step's y is dropped by the reference
KFP = 16                  # fixed-point iterations per time step
CH = 64                   # time steps per DMA chunk

F32 = mybir.dt.float32
F32R = mybir.dt.float32r


def _host_params(x0_sys, X, Y, B2, C2, D21, D22, D12):
    n, q = N_STATE, Q
    X = np.asarray(X, np.float64)
    Y = np.asarray(Y, np.float64)
    B2 = np.asarray(B2, np.float64)
    C2 = np.asarray(C2, np.float64)
    D21 = np.asarray(D21, np.float64)
    D22 = np.asarray(D22, np.float64)
    D12 = np.asarray(D12, np.float64)

    H = X.T @ X + EPS * np.eye(2 * n + q)
    H11 = H[:n, :n]
    H21 = H[n:n + q, :n]
    H22 = H[n:n + q, n:n + q]
    H31 = H[n + q:, :n]
    H32 = H[n + q:, n:n + q]
    H33 = H[n + q:, n + q:]
    F_ = H31
    B1 = H32
    E_inv = np.linalg.inv(0.5 * (H11 + ALPHA * H33 + Y - Y.T))
    Lam = 0.5 * np.diag(H22)
    D11 = -np.tril(H22, -1)
    C1 = -H21

    FE = E_inv @ F_
    B1E = E_inv @ B1
    B2E = E_inv @ B2
    C1t = C1 / Lam[:, None]
    D12t = D12 / Lam[:, None]

    f32 = lambda a: np.ascontiguousarray(a, np.float32)
    # lhsT layouts (pre-transposed for the tensor engine: out = lhsT.T @ rhs)
    params = {
        "W_Dt": f32((D11 / Lam[:, None]).T),        # (q, q)
        "W_C1t": f32(C1t.T),                        # (n, q)   step 0 only
        "W_D12t": f32(D12t.T),                      # (in, q)
        "W_AX": f32((C1t @ FE).T),                  # (n, q)
        "W_AW": f32((C1t @ B1E).T),                 # (q, q)
        "W_AU0": f32((C1t @ B2E).T),                # (in, q)
        "W_FE": f32(FE.T),                          # (n, n)
        "W_B1E": f32(B1E.T),                        # (q, n)
        "W_B2E": f32(B2E.T),                        # (in, n)
        "W_YX": f32((C2 @ FE).T),                   # (n, out)
        "W_YW": f32((C2 @ B1E + D21).T),            # (q, out)
        "W_YU": f32((C2 @ B2E + D22).T),            # (in, out)
    }

    y0_sys = np.asarray(x0_sys, np.float64)[:, 0, :]       # (B, out)
    x0 = (np.linalg.pinv(C2) @ y0_sys.T).T                 # (B, n)
    y0 = x0 @ C2.T                                         # (B, out)
    return params, f32(x0), f32(y0)


_W_SHAPES = [
    ("W_Dt", (Q, Q)),
    ("W_C1t", (N_STATE, Q)),
    ("W_D12t", (IN_DIM, Q)),
    ("W_AX", (N_STATE, Q)),
    ("W_AW", (Q, Q)),
    ("W_AU0", (IN_DIM, Q)),
    ("W_FE", (N_STATE, N_STATE)),
    ("W_B1E", (Q, N_STATE)),
    ("W_B2E", (IN_DIM, N_STATE)),
    ("W_YX", (N_STATE, OUT_DIM)),
    ("W_YW", (Q, OUT_DIM)),
    ("W_YU", (IN_DIM, OUT_DIM)),
]


def _build():
    """Build + compile the single-core program (identical on all cores)."""
    nc = bacc.Bacc(
        "TRN2", target_bir_lowering=False, debug=False, enable_asserts=True
    )
    u_d = nc.dram_tensor("u", (IN_DIM, NSTEP, BL), F32, kind="ExternalInput").ap()
    x0_d = nc.dram_tensor("x0", (N_STATE, BL), F32, kind="ExternalInput").ap()
    wd = {
        name: nc.dram_tensor(name, shape, F32, kind="ExternalInput").ap()
        for name, shape in _W_SHAPES
    }
    y_d = nc.dram_tensor("y", (OUT_DIM, NSTEP, BL), F32, kind="ExternalOutput").ap()

    Tanh = mybir.ActivationFunctionType.Tanh
    n_chunks = (NSTEP + CH - 1) // CH
    def mm(out, w_tile, rhs, start, stop):
        nc.tensor.matmul(out[:], w_tile[:], rhs, start=start, stop=stop)

    def mm_ct(out, w_tile, rhs):
        nc.tensor.matmul(out[:], w_tile[:], rhs, start=False, stop=True)

    with tile.TileContext(nc) as tc:
        with (
            tc.tile_pool(name="singles", bufs=1) as singles,
            tc.tile_pool(name="xp", bufs=3) as xp,
            tc.tile_pool(name="wp", bufs=8) as wp,
            tc.tile_pool(name="yo", bufs=2) as yo,
            tc.tile_pool(name="fp", bufs=5, space="PSUM") as fp_pool,
            tc.tile_pool(name="px", bufs=1, space="PSUM") as px_pool,
            tc.tile_pool(name="py", bufs=1, space="PSUM") as py_pool,
        ):
            # --- load constants ---
            w_sb = {}
            for name, d in wd.items():
                t_ = singles.tile(list(d.shape), F32, tag=name)
                nc.sync.dma_start(t_[:], d[:])
                w_sb[name] = t_

            # --- load the whole u trajectory (chunked so compute can start) ---
            u_sb = singles.tile([IN_DIM, NSTEP, BL], F32, tag="u_sb")
            for c in range(n_chunks):
                c0, c1 = c * CH, min((c + 1) * CH, NSTEP)
                nc.sync.dma_start(u_sb[:, c0:c1, :], u_d[:, c0:c1, :])

            x_cur = xp.tile([N_STATE, BL], F32, tag="x")
            nc.sync.dma_start(x_cur[:], x0_d[:])

            # Pipeline discipline: at the START of step t's body,
            #   x_ready = x_{t-1} (most recent materialized state)
            #   w_fin   = w_{t-1} (final w of the previous step)
            #   pa      = at-bank for step t with the u/x terms already
            #             accumulated (emitted during step t-1)
            # Tile schedules the PE stream statically in emission order, so
            # every off-chain matmul is emitted in an iteration slot of the
            # step where its inputs become ready; only the AW hop (which
            # needs w_{t-1}) sits at the step boundary.  w_fin readers sit in
            # the first few slots to stay clear of the w-pool WAR horizon.
            x_ready = x_cur   # x0
            w_fin = None
            pa_next = None
            chunk_tiles = {}
            for c in range(n_chunks):
                c0, c1 = c * CH, min((c + 1) * CH, NSTEP)
                chunk_tiles[c] = yo.tile([OUT_DIM, CH, BL], F32, tag="y_chunk",
                                         name="y_chunk")
                for t in range(c0, c1):
                    u_t = u_sb[:, t, :]
                    # at = Lam^-1 (C1 x_t + D12 u_t), refolded for t>0 so the
                    # only chain input is w_{t-1}
                    if t == 0:
                        pa = fp_pool.tile([Q, BL], F32, tag="fp", name="pa")
                        mm(pa, w_sb["W_D12t"], u_t, True, False)
                        mm(pa, w_sb["W_C1t"], x_ready[:], False, True)
                    else:
                        pa = pa_next
                        mm_ct(pa, w_sb["W_AW"], w_fin[:])
                    w_cur = wp.tile([Q, BL], F32, tag="w")
                    nc.scalar.activation(w_cur[:], pa[:], Tanh)
                    # deferred work, one logical op per iteration slot:
                    #  - y/x update of step t-1 (needs w_{t-1}, x_{t-1})
                    #  - u/x terms of at for step t+1 (needs x_t from slot 8)
                    todo = []
                    x_nxt = None
                    if t > 0:
                        tp = t - 1
                        py = py_pool.tile([OUT_DIM, BL], F32, tag="py",
                                          name="py")
                        px = px_pool.tile([N_STATE, BL], F32, tag="px",
                                          name="px")
                        u_d1 = u_sb[:, tp, :]
                        cp = tp // CH
                        yck = chunk_tiles[cp]
                        x_nxt = xp.tile([N_STATE, BL], F32, tag="x",
                                        name="x_nxt")
                        xr, wf = x_ready, w_fin
                        ce = min((cp + 1) * CH, NSTEP) - 1
                        todo += [
                            lambda: mm(py, w_sb["W_YU"], u_d1, True, False),
                            lambda: mm(py, w_sb["W_YX"], xr[:], False, False),
                            lambda: mm(px, w_sb["W_B2E"], u_d1, True, False),
                            lambda: mm(px, w_sb["W_FE"], xr[:], False, False),
                            lambda: mm(py, w_sb["W_YW"], wf[:], False, True),
                            lambda: mm(px, w_sb["W_B1E"], wf[:], False, True),
                            lambda: nc.vector.tensor_copy(
                                yck[:, tp - cp * CH, :], py[:]),
                            lambda: nc.vector.tensor_copy(x_nxt[:], px[:]),
                            lambda: nc.sync.dma_start(
                                y_d[:, cp * CH:tp + 1, :],
                                yck[:, : tp + 1 - cp * CH, :])
                            if tp == ce else None,
                        ]
                    else:
                        todo += [None] * 9
                    if t < NSTEP - 1:
                        pa_next = fp_pool.tile([Q, BL], F32, tag="fp",
                                               name="pa_next")
                        pn = pa_next
                        u_n = u_sb[:, t + 1, :]
                        xn = x_nxt if x_nxt is not None else x_ready
                        todo += [
                            lambda: mm(pn, w_sb["W_D12t"], u_n, True, False),
                            lambda: mm(pn, w_sb["W_AU0"], u_t, False, False),
                            lambda: mm(pn, w_sb["W_AX"], xn[:], False, False),
                        ]
                    # fixed-point iterations: w <- tanh(at + Dt w).
                    # Prefill each bank with `at` via DVE (PSUM->PSUM copy);
                    # walrus inserts the has_written workaround so the chain
                    # matmuls accumulate on top.
                    for it in range(1, KFP):
                        pm = fp_pool.tile([Q, BL], F32, tag="fp", name="pm")
                        nc.vector.tensor_copy(pm[:], pa[:])
                        mm_ct(pm, w_sb["W_Dt"], w_cur[:])
                        if it - 1 < len(todo) and todo[it - 1] is not None:
                            todo[it - 1]()
                        w_nxt = wp.tile([Q, BL], F32, tag="w")
                        nc.scalar.activation(w_nxt[:], pm[:], Tanh)
                        w_cur = w_nxt
                    for fn in todo[KFP - 1:]:
                        if fn is not None:
                            fn()
                    if x_nxt is not None:
                        x_ready = x_nxt
                    w_fin = w_cur
            # last step: nothing defers it, flush inline
            tp = NSTEP - 1
            py = py_pool.tile([OUT_DIM, BL], F32, tag="py", name="py")
            u_d1 = u_sb[:, tp, :]
            cp = tp // CH
            yck = chunk_tiles[cp]
            mm(py, w_sb["W_YU"], u_d1, True, False)
            mm(py, w_sb["W_YX"], x_ready[:], False, False)
            mm(py, w_sb["W_YW"], w_fin[:], False, True)
            nc.vector.tensor_copy(yck[:, tp - cp * CH, :], py[:])
            nc.sync.dma_start(
                y_d[:, cp * CH:tp + 1, :], yck[:, : tp + 1 - cp * CH, :])

    nc.compile()
    return nc


_NC_CACHE = []


def _get_nc():
    if not _NC_CACHE:
        _NC_CACHE.append(_build())
    return _NC_CACHE[0]


def _run(inputs, **spmd_kwargs):
    params, x0, y0 = _host_params(
        inputs["x0_sys"], inputs["X"], inputs["Y"], inputs["B2"],
        inputs["C2"], inputs["D21"], inputs["D22"], inputs["D12"],
    )
    u_in = np.ascontiguousarray(inputs["u_in"], np.float32)

    nc = _get_nc()
    in_maps = []
    for s in range(NCORES):
        b0, b1 = s * BL, (s + 1) * BL
        m = dict(params)
        # (BL, NSTEP, IN) -> (IN, NSTEP, BL)
        m["u"] = np.ascontiguousarray(u_in[b0:b1, :NSTEP, :].transpose(2, 1, 0))
        m["x0"] = np.ascontiguousarray(x0[b0:b1].T)
        in_maps.append(m)

    res = run_bass_kernel_spmd(nc, in_maps, list(range(NCORES)), **spmd_kwargs)

    out = np.empty((B, T, OUT_DIM), np.float32)
    out[:, 0, :] = y0
    for s in range(NCORES):
        b0, b1 = s * BL, (s + 1) * BL
        # (OUT, NSTEP, BL) -> (BL, NSTEP, OUT)
        out[b0:b1, 1:, :] = res.results[s]["y"].transpose(2, 1, 0)
    return out, res


def kernel(**inputs) -> np.ndarray:
    out, _ = _run(inputs)
    return out
